# revision 9
# baseline (speedup 1.0000x reference)
"""FSMN memory block (strided dilated depthwise conv over time) on 8 trn2 cores.

out[b,t,d] = sum_k filt[k,d] * x[b, t + off_k - 20, d] + x[b,t,d]
  off_k in {0,2,..,18} (left), {20} (center), {21,23,..,29} (right)

Architecture (v5):
- Data-parallel over batch: 16 items -> 2 per core, identical SPMD program.
- Host pre-transposes to channel-major [b, d, t] bf16 with zero time-padding
  so every DMA row is contiguous. 8 rounds per core: (g, b) for 4 channel
  groups x 2 batch items, tiles [128, 2032] (per-partition pitch <= 2047
  elements -- REQUIRED: DVE 2x/4x perf-mode uops can't encode larger
  pitches and silently fall back to 1x/2x).
- The 16 taps are split by engine throughput:
    * PE (307 GMAC/s diag): 10 taps as diag-weight bf16 matmuls, 4 psum
      chunks of 500 cols, tap-outer so LDWEIGHTS dedupes. Rounds alternate
      between two 4-bank PSUM sets so round r's matmuls never wait on
      round r-1's merges.
    * DVE: 2 taps (tensor_scalar 4x mult + tensor_tensor 2x add), the
      per-chunk psum merge out = psum + acc (scalar_tensor_tensor), one
      partial-fold add, and the final acc += pa0.
    * Act (1 elem/cyc): 4 taps as per-partition-scaled copies pa0..pa3.
    * Folds: pa2 += pa3 via SWDGE accum-DMA (gpsimd trigger), pa0 += pa1
      on Pool tensor_tensor, pa0 += pa2 and acc += pa0 on DVE.
- Residual folded into the center tap weight (1+f) on PE.
- One [128, 2000] bf16 store per round (4000B descriptors).
- PE p-state warmup: ~34 dependency-free junk matmuls reading a memset tile
  keep the PE clock ramping from t=0 (full clock needs 3us continuous busy).
"""

import sys

for p in ("/opt/trn_rl_repo", "/opt/trn_rl_repo/concourse"):
    if p not in sys.path:
        sys.path.insert(0, p)

import ml_dtypes
import numpy as np

import concourse.bass as bass
import concourse.mybir as mybir
from concourse.bass_utils import run_bass_kernel_spmd
from concourse.tile import TileContext

# Problem constants (hardcoded per contract).
B, T, D = 16, 2000, 512
NCORES = 8
B_LOC = B // NCORES          # 2 batch items per core
P = 128                      # partitions
NG = D // P                  # 4 channel groups
NTAPS = 16
OFFS = [2 * k for k in range(10)] + [20] + [21 + 2 * k for k in range(5)]
PADL = 20                    # left zero pad inside the padded time axis
TP = T + 32                  # padded time per batch block (20 + 2000 + 12)
CH = 500                     # psum chunk (one bank holds 512 fp32)
NCHK = T // CH               # 4 chunks per round
F32 = mybir.dt.float32
BF16 = mybir.dt.bfloat16
NPBF16 = ml_dtypes.bfloat16

# Engine tap assignment (indices into OFFS/filt rows). Center tap (10)
# carries the residual; keep it on PE (fp32 psum accumulation).
DVE_TAPS = [0, 1]
ACT_TAPS = [2, 3, 4, 5]
PE_TAPS = [k for k in range(NTAPS) if k not in DVE_TAPS and k not in ACT_TAPS]
NV = len(DVE_TAPS) + len(ACT_TAPS)   # 6 scalar-filter slots
NPE = len(PE_TAPS)                   # 10
NJUNK = 34                           # p-state warmup matmuls
ROUNDS = [(g, b) for g in range(NG) for b in range(B_LOC)]  # g-major

_CACHE = {}


def _build_bass(waitfix: bool = True):
    nc = bass.Bass()
    x = nc.declare_dram_parameter("x", [B_LOC, D, TP], BF16, isOutput=False)
    dw = nc.declare_dram_parameter("dw", [P, NG, NPE, P], BF16, isOutput=False)
    fv = nc.declare_dram_parameter("fv", [P, NV, NG], F32, isOutput=False)
    youts = {
        (b, g): nc.declare_dram_parameter(
            f"y_{b}_{g}", [P, T], BF16, isOutput=True
        )
        for b in range(B_LOC)
        for g in range(NG)
    }

    with TileContext(nc) as tc:
        with (
            tc.tile_pool(name="wpool", bufs=1) as wpool,
            tc.tile_pool(name="xpool", bufs=len(ROUNDS)) as xpool,
            tc.tile_pool(name="pap", bufs=2) as pa_pool,
            tc.tile_pool(name="accp", bufs=2) as acc_pool,
            tc.tile_pool(name="outp", bufs=2) as out_pool,
            tc.tile_pool(name="psum", bufs=8, space="PSUM") as ps_pool,
        ):
            # Tiny junk-weight tile, engine-memset so the warmup matmuls have
            # no DMA dependency and can start at t=0.
            junkin = wpool.tile([P, 128], BF16, name="junkin")
            nc.vector.memset(junkin[0:1, :], 0.0)

            fvt = wpool.tile([P, NV, NG], F32, name="fvt")
            nc.sync.dma_start(out=fvt, in_=fv[:, :, :])

            xts = {}
            for r in range(len(ROUNDS)):
                xts[r] = xpool.tile([P, TP], BF16, name="xt")
            wt = wpool.tile([P, NG, NPE, P], BF16, name="wt")

            def load_xt(r, segs=(TP,)):
                g, b = ROUNDS[r]
                lo = 0
                for hi in segs:
                    nc.sync.dma_start(
                        out=xts[r][:, lo:hi],
                        in_=x[b, g * P : (g + 1) * P, lo:hi],
                    )
                    lo = hi

            # Each dma_start is pinned to ONE DMA queue (~24 GB/s), so the
            # latency-critical first-round data is split across several
            # queues: 4 segments of xt0 and per-tap-group pieces of the g=0
            # weights (the first LDWEIGHTS needs only taps 0-1).
            nc.sync.dma_start(out=wt[:, 0, 0:2], in_=dw[:, 0, 0:2])
            load_xt(0, segs=(536, 1036, 1536, TP))
            nc.sync.dma_start(out=wt[:, 0, 2:6], in_=dw[:, 0, 2:6])
            nc.sync.dma_start(out=wt[:, 0, 6:NPE], in_=dw[:, 0, 6:NPE])
            load_xt(1, segs=(1036, TP))
            nc.sync.dma_start(out=wt[:, 1, 0:5], in_=dw[:, 1, 0:5])
            load_xt(2)
            nc.sync.dma_start(out=wt[:, 1, 5:NPE], in_=dw[:, 1, 5:NPE])
            load_xt(3)
            nc.sync.dma_start(out=wt[:, 2, 0:5], in_=dw[:, 2, 0:5])
            load_xt(4)
            nc.sync.dma_start(out=wt[:, 2, 5:NPE], in_=dw[:, 2, 5:NPE])
            load_xt(5)
            nc.sync.dma_start(out=wt[:, 3, 0:5], in_=dw[:, 3, 0:5])
            load_xt(6)
            nc.sync.dma_start(out=wt[:, 3, 5:NPE], in_=dw[:, 3, 5:NPE])
            load_xt(7)

            # PE p-state warmup: dependency-free junk stream from t=0. Writes
            # land in round-0/1 psum tiles; the real start=True matmuls reset
            # them (the WAW dep just orders junk before the real use).
            pss_r = {}
            for r in range(len(ROUNDS)):
                pss_r[r] = [
                    ps_pool.tile([P, CH], F32, name="ps") for _ in range(NCHK)
                ]
            for j in range(NJUNK):
                nc.tensor.matmul(
                    pss_r[0][j % 2][0:1, 0:64],
                    junkin[0:1, 0:1],
                    junkin[0:1, 0:64],
                    start=True, stop=True, skip_group_check=True,
                )

            for r, (g, b) in enumerate(ROUNDS):
                xt = xts[r]

                # ---- Act taps: per-partition-scaled copies (bf16) ----
                pas = []
                for ai, k in enumerate(ACT_TAPS):
                    vi = len(DVE_TAPS) + ai
                    pa = pa_pool.tile([P, T], BF16, name=f"pa{ai}")
                    nc.scalar.mul(
                        pa, xt[:, OFFS[k] : OFFS[k] + T], fvt[:, vi, g : g + 1]
                    )
                    pas.append(pa)

                # Folds: pa0 += pa1 and pa2 += pa3 on the DMA fabric (SWDGE
                # accum, gpsimd trigger only -- NO Pool compute: Pool
                # tensor_tensor holds the DVE/GpSimd shared SBUF port pair
                # for the whole instruction and starves DVE's 2-port perf
                # modes). The remaining folds are cheap 2x-mode DVE adds.
                nc.gpsimd.dma_start(
                    out=pas[0], in_=pas[1], accum_op=mybir.AluOpType.add
                )
                nc.gpsimd.dma_start(
                    out=pas[2], in_=pas[3], accum_op=mybir.AluOpType.add
                )

                # ---- DVE taps: 4x-mode mult + 2x-mode adds, bf16 ----
                acc = acc_pool.tile([P, T], BF16, name="acc")
                tmp = acc_pool.tile([P, T], BF16, name="tmp")
                for vi, k in enumerate(DVE_TAPS):
                    w = xt[:, OFFS[k] : OFFS[k] + T]
                    if vi == 0:
                        nc.vector.tensor_scalar(
                            acc, w, fvt[:, vi, g : g + 1], None,
                            mybir.AluOpType.mult,
                        )
                    else:
                        nc.vector.tensor_scalar(
                            tmp, w, fvt[:, vi, g : g + 1], None,
                            mybir.AluOpType.mult,
                        )
                        nc.vector.tensor_tensor(
                            acc, acc, tmp, mybir.AluOpType.add
                        )
                nc.vector.tensor_tensor(
                    acc, acc, pas[0], mybir.AluOpType.add
                )
                nc.vector.tensor_tensor(
                    acc, acc, pas[2], mybir.AluOpType.add
                )

                # ---- PE taps: tap-outer over this round's 4-bank psum set
                # (sets alternate per round) so LDWEIGHTS dedupes and the
                # previous round's banks are already merged. ----
                pss = pss_r[r]
                out_sb = out_pool.tile([P, T], BF16, name="out_sb")
                for n_, (ki, k) in enumerate(enumerate(PE_TAPS)):
                    for c in range(NCHK):
                        nc.tensor.matmul(
                            pss[c],
                            wt[:, g, ki, :],
                            xt[:, c * CH + OFFS[k] : c * CH + OFFS[k] + CH],
                            start=(n_ == 0),
                            stop=(n_ == NPE - 1),
                            skip_group_check=True,
                        )
                # ---- merge: out = psum + acc per chunk on DVE, store.
                # Last round stores per chunk-pair to shorten the tail. ----
                last = r == len(ROUNDS) - 1
                for c in range(NCHK):
                    sl = slice(c * CH, (c + 1) * CH)
                    nc.vector.scalar_tensor_tensor(
                        out_sb[:, sl], pss[c], 1.0, acc[:, sl],
                        mybir.AluOpType.mult, mybir.AluOpType.add,
                    )
                    if last and c % 2 == 1:
                        nc.sync.dma_start(
                            out=youts[(b, g)][:, c * CH - CH : (c + 1) * CH],
                            in_=out_sb[:, c * CH - CH : (c + 1) * CH],
                        )
                if not last:
                    nc.sync.dma_start(out=youts[(b, g)][:, :], in_=out_sb[:, :])

    # The tile legalizer emits one LDWEIGHTS per bf16 matmul; with tap-outer
    # ordering the 4 chunk matmuls of one tap reload identical weights.
    # Drop the duplicates, migrating their waits to the next PE-queue
    # instruction.
    PE_ENG = mybir.EngineType.PE
    for fn in nc.m.functions:
        for blk in fn.blocks:
            out_insts = []
            last_key = None
            pending = []
            for inst in blk.instructions:
                tn = type(inst).__name__
                if getattr(inst, "engine", None) == PE_ENG or tn in (
                    "InstLdweights",
                    "InstMatmult",
                ):
                    if tn == "InstLdweights":
                        w = inst.ins[0]
                        key = (
                            w.memref,
                            w.offset,
                            str(w.ap),
                            str(w.dtype),
                            str(inst.perf_mode),
                            str(inst.is_transpose),
                        )
                        if key == last_key:
                            si = inst.sync_info
                            if si is not None:
                                assert not si.on_update, inst.name
                                pending.extend(si.on_wait)
                            continue  # drop duplicate
                        last_key = key
                    elif tn == "InstMatmult":
                        pass  # legalized matmuls don't clobber the array
                    else:
                        last_key = None  # unknown PE op: be conservative
                    if pending:
                        si = inst.sync_info
                        ow = list(si.on_wait) if si else []
                        ou = list(si.on_update) if si else []
                        inst.sync_info = mybir.SyncInfo(
                            on_wait=pending + ow, on_update=ou
                        )
                        pending = []
                out_insts.append(inst)
            assert not pending
            blk.instructions = out_insts

    # TRN2 ISA structs encode a single sync-wait. Split every multi-wait
    # instruction: single-wait NoOps on the same queue immediately before it
    # carry the extra waits (the sequencer blocks on each in order).
    if not waitfix:
        return nc
    nfix = [0]
    for fn in nc.m.functions:
        for blk in fn.blocks:
            out_insts = []
            for inst in blk.instructions:
                si = inst.sync_info
                if si is not None and len(si.on_wait) > 1:
                    w = list(si.on_wait)
                    for wt_ in w[:-1]:
                        nop = mybir.InstNoOp(name=f"waitfix_{nfix[0]}")
                        nfix[0] += 1
                        nop.engine = inst.engine
                        nop.sync_info = mybir.SyncInfo(
                            on_wait=[wt_], on_update=[]
                        )
                        out_insts.append(nop)
                    inst.sync_info = mybir.SyncInfo(
                        on_wait=[w[-1]], on_update=list(si.on_update)
                    )
                out_insts.append(inst)
            blk.instructions = out_insts
    return nc


def _pack_weights(filt: np.ndarray):
    fw = filt.astype(np.float32).copy()
    fw[10] += 1.0  # fold the residual into the center tap (on PE)
    dwm = np.zeros((P, NG, NPE, P), np.float32)
    for ki, k in enumerate(PE_TAPS):
        for g in range(NG):
            dwm[np.arange(P), g, ki, np.arange(P)] = fw[k, g * P : (g + 1) * P]
    fvm = np.zeros((P, NV, NG), np.float32)
    for vi, k in enumerate(DVE_TAPS + ACT_TAPS):
        for g in range(NG):
            fvm[:, vi, g] = fw[k, g * P : (g + 1) * P]
    return dwm.astype(NPBF16), fvm


def kernel(inputs: np.ndarray, filt: np.ndarray, _trace: bool = False):
    inputs = np.asarray(inputs, dtype=np.float32)
    filt = np.asarray(filt, dtype=np.float32)

    # Channel-major, zero-padded, bf16.
    xp = np.zeros((B, D, TP), NPBF16)
    xp[:, :, PADL : PADL + T] = inputs.transpose(0, 2, 1).astype(NPBF16)
    dwm, fvm = _pack_weights(filt)
    in_maps = [
        {"x": xp[c * B_LOC : (c + 1) * B_LOC], "dw": dwm, "fv": fvm}
        for c in range(NCORES)
    ]

    if "nc" not in _CACHE:
        _CACHE["nc"] = _build_bass()
    nc = _CACHE["nc"]
    res = run_bass_kernel_spmd(nc, in_maps, list(range(NCORES)), trace=_trace)
    ycm = np.empty((B, D, T), NPBF16)
    for core in range(NCORES):
        r = res.results[core]
        for b in range(B_LOC):
            for g in range(NG):
                ycm[core * B_LOC + b, g * P : (g + 1) * P, :] = np.asarray(
                    r[f"y_{b}_{g}"]
                )
    out = np.ascontiguousarray(ycm.transpose(0, 2, 1)).astype(np.float32)
    if _trace:
        return out, res
    return out


if __name__ == "__main__":
    rng = np.random.default_rng(0)
    xs = rng.standard_normal((B, T, D), dtype=np.float32)
    ft = rng.standard_normal((NTAPS, D), dtype=np.float32)
    out = kernel(xs, ft)
    print("ran ok", out.shape, out.dtype)


# revision 19
# speedup vs baseline: 1.1262x; 1.1262x over previous
"""FSMN memory block (strided dilated depthwise conv over time) on 8 trn2 cores.

out[b,t,d] = sum_k filt[k,d] * x[b, t + off_k - 20, d] + x[b,t,d]
  off_k in {0,2,..,18} (left), {20} (center), {21,23,..,29} (right)

Architecture (v8):
- Data-parallel over batch: 16 items -> 2 per core, identical SPMD program.
- Host pre-transposes to channel-major [b, d, t] bf16 with zero time-padding
  so every DMA row is contiguous. 8 rounds per core: (g, b) for 4 channel
  groups x 2 batch items, tiles [128, 2032] (per-partition pitch <= 2047
  elements -- REQUIRED: DVE 2x/4x perf-mode uops can't encode larger
  pitches and silently fall back to 1x/2x).
- The 16 taps are split by engine throughput:
    * PE (307 GMAC/s diag): 10 taps as diag-weight bf16 matmuls, 4 psum
      chunks of 500 cols, tap-outer so LDWEIGHTS dedupes (and is fully
      hidden behind the column stream). Rounds alternate between two
      4-bank PSUM sets so round r's matmuls never wait on round r-1's
      merges.
    * DVE: 2 taps (tensor_scalar 4x mult + tensor_tensor 2x add), the
      per-chunk psum merge out = psum + acc (scalar_tensor_tensor), and
      two partial-fold adds.
    * Act (1 elem/cyc, own SBUF port): 4 taps as per-partition-scaled
      copies pa0..pa3.
    * Folds: pa0 += pa1 and pa2 += pa3 via SWDGE accum-DMAs (gpsimd
      trigger; NO Pool compute -- Pool tensor_tensor would hold the
      DVE/GpSimd shared SBUF port pair and starve DVE's perf modes);
      acc += pa0, acc += pa2 on DVE. Last round folds on DVE instead to
      shorten the kernel tail.
- Residual folded into the center tap weight (1+f) on PE.
- One [128, 2000] bf16 store per round (4000B descriptors, one DMA queue
  each -- measured faster than striping the store across queues).
- Loads: each dma_start is pinned to one ~24 GB/s DMA queue, so the
  first-round data and per-tap-group weight pieces are split across
  several queues; everything lands before the ~7us framework preamble
  releases the engines, so no PE warmup is needed.
"""

import sys

for p in ("/opt/trn_rl_repo", "/opt/trn_rl_repo/concourse"):
    if p not in sys.path:
        sys.path.insert(0, p)

import ml_dtypes
import numpy as np

import concourse.bass as bass
import concourse.mybir as mybir
from concourse.bass_utils import run_bass_kernel_spmd
from concourse.tile import TileContext

# Problem constants (hardcoded per contract).
B, T, D = 16, 2000, 512
NCORES = 8
B_LOC = B // NCORES          # 2 batch items per core
P = 128                      # partitions
NG = D // P                  # 4 channel groups
NTAPS = 16
OFFS = [2 * k for k in range(10)] + [20] + [21 + 2 * k for k in range(5)]
PADL = 20                    # left zero pad inside the padded time axis
TP = T + 32                  # padded time per batch block (20 + 2000 + 12)
CH = 500                     # psum chunk (one bank holds 512 fp32)
NCHK = T // CH               # 4 chunks per round
F32 = mybir.dt.float32
BF16 = mybir.dt.bfloat16
NPBF16 = ml_dtypes.bfloat16

# Engine tap assignment (indices into OFFS/filt rows). Center tap (10)
# carries the residual; keep it on PE (fp32 psum accumulation).
DVE_TAPS = [0, 1]
ACT_TAPS = [2, 3, 4, 5]
PE_TAPS = [k for k in range(NTAPS) if k not in DVE_TAPS and k not in ACT_TAPS]
NV = len(DVE_TAPS) + len(ACT_TAPS)   # 6 scalar-filter slots
NPE = len(PE_TAPS)                   # 10
ROUNDS = [(g, b) for g in range(NG) for b in range(B_LOC)]  # g-major

_CACHE = {}
_VARIANT = {}  # build options override, e.g. {'junk': 34}


def _build_bass(waitfix: bool = True, junk: int = 0, last_dve_folds: bool = True,
                store_split: bool = False):
    nc = bass.Bass()
    x = nc.declare_dram_parameter("x", [B_LOC, D, TP], BF16, isOutput=False)
    dw = nc.declare_dram_parameter("dw", [P, NG, NPE, P], BF16, isOutput=False)
    fv = nc.declare_dram_parameter("fv", [P, NV, NG], F32, isOutput=False)
    youts = {
        (b, g): nc.declare_dram_parameter(
            f"y_{b}_{g}", [P, T], BF16, isOutput=True
        )
        for b in range(B_LOC)
        for g in range(NG)
    }

    with TileContext(nc) as tc:
        with (
            tc.tile_pool(name="wpool", bufs=1) as wpool,
            tc.tile_pool(name="xpool", bufs=len(ROUNDS)) as xpool,
            tc.tile_pool(name="pap", bufs=2) as pa_pool,
            tc.tile_pool(name="accp", bufs=2) as acc_pool,
            tc.tile_pool(name="outp", bufs=3) as out_pool,
            tc.tile_pool(name="psum", bufs=8, space="PSUM") as ps_pool,
        ):
            fvt = wpool.tile([P, NV, NG], F32, name="fvt")
            nc.sync.dma_start(out=fvt, in_=fv[:, :, :])

            xts = {}
            for r in range(len(ROUNDS)):
                xts[r] = xpool.tile([P, TP], BF16, name="xt")
            wt = wpool.tile([P, NG, NPE, P], BF16, name="wt")

            def load_xt(r, segs=(TP,)):
                g, b = ROUNDS[r]
                lo = 0
                for hi in segs:
                    nc.sync.dma_start(
                        out=xts[r][:, lo:hi],
                        in_=x[b, g * P : (g + 1) * P, lo:hi],
                    )
                    lo = hi

            # Each dma_start is pinned to ONE DMA queue (~24 GB/s), so the
            # latency-critical first-round data is split across several
            # queues: 4 segments of xt0 and per-tap-group pieces of the g=0
            # weights (the first LDWEIGHTS needs only taps 0-1).
            nc.sync.dma_start(out=wt[:, 0, 0:2], in_=dw[:, 0, 0:2])
            load_xt(0, segs=(536, 1036, 1536, TP))
            nc.sync.dma_start(out=wt[:, 0, 2:6], in_=dw[:, 0, 2:6])
            nc.sync.dma_start(out=wt[:, 0, 6:NPE], in_=dw[:, 0, 6:NPE])
            load_xt(1, segs=(1036, TP))
            nc.sync.dma_start(out=wt[:, 1, 0:5], in_=dw[:, 1, 0:5])
            load_xt(2)
            nc.sync.dma_start(out=wt[:, 1, 5:NPE], in_=dw[:, 1, 5:NPE])
            load_xt(3)
            nc.sync.dma_start(out=wt[:, 2, 0:5], in_=dw[:, 2, 0:5])
            load_xt(4)
            nc.sync.dma_start(out=wt[:, 2, 5:NPE], in_=dw[:, 2, 5:NPE])
            load_xt(5)
            nc.sync.dma_start(out=wt[:, 3, 0:5], in_=dw[:, 3, 0:5])
            load_xt(6)
            nc.sync.dma_start(out=wt[:, 3, 5:NPE], in_=dw[:, 3, 5:NPE])
            load_xt(7)

            # (The framework preamble holds all engines until ~7us, by
            # which time the split loads above have landed; optional junk
            # matmuls keep the PE p-state ramping if enabled.)
            pss_r = {}
            for r in range(len(ROUNDS)):
                pss_r[r] = [
                    ps_pool.tile([P, CH], F32, name="ps") for _ in range(NCHK)
                ]
            if junk:
                junkin = wpool.tile([P, 128], BF16, name="junkin")
                nc.vector.memset(junkin[0:1, :], 0.0)
                for j in range(junk):
                    nc.tensor.matmul(
                        pss_r[0][j % 2][0:1, 0:64],
                        junkin[0:1, 0:1],
                        junkin[0:1, 0:64],
                        start=True, stop=True, skip_group_check=True,
                    )

            for r, (g, b) in enumerate(ROUNDS):
                xt = xts[r]

                # ---- Act taps: per-partition-scaled copies (bf16) ----
                pas = []
                for ai, k in enumerate(ACT_TAPS):
                    vi = len(DVE_TAPS) + ai
                    pa = pa_pool.tile([P, T], BF16, name=f"pa{ai}")
                    nc.scalar.mul(
                        pa, xt[:, OFFS[k] : OFFS[k] + T], fvt[:, vi, g : g + 1]
                    )
                    pas.append(pa)

                # Folds: pa0 += pa1 and pa2 += pa3 on the DMA fabric (SWDGE
                # accum, gpsimd trigger only -- NO Pool compute: Pool
                # tensor_tensor holds the DVE/GpSimd shared SBUF port pair
                # for the whole instruction and starves DVE's 2-port perf
                # modes). The remaining folds are cheap 2x-mode DVE adds.
                # Last round: folds on DVE instead -- the SWDGE queues are
                # deep by then and would stretch the kernel tail.
                last = r == len(ROUNDS) - 1
                if not (last and last_dve_folds):
                    nc.gpsimd.dma_start(
                        out=pas[0], in_=pas[1], accum_op=mybir.AluOpType.add
                    )
                    nc.gpsimd.dma_start(
                        out=pas[2], in_=pas[3], accum_op=mybir.AluOpType.add
                    )

                # ---- DVE taps: 4x-mode mult + 2x-mode adds, bf16 ----
                acc = acc_pool.tile([P, T], BF16, name="acc")
                tmp = acc_pool.tile([P, T], BF16, name="tmp")
                for vi, k in enumerate(DVE_TAPS):
                    w = xt[:, OFFS[k] : OFFS[k] + T]
                    if vi == 0:
                        nc.vector.tensor_scalar(
                            acc, w, fvt[:, vi, g : g + 1], None,
                            mybir.AluOpType.mult,
                        )
                    else:
                        nc.vector.tensor_scalar(
                            tmp, w, fvt[:, vi, g : g + 1], None,
                            mybir.AluOpType.mult,
                        )
                        nc.vector.tensor_tensor(
                            acc, acc, tmp, mybir.AluOpType.add
                        )
                if last and last_dve_folds:
                    nc.vector.tensor_tensor(
                        pas[0], pas[0], pas[1], mybir.AluOpType.add
                    )
                    nc.vector.tensor_tensor(
                        pas[2], pas[2], pas[3], mybir.AluOpType.add
                    )
                nc.vector.tensor_tensor(
                    acc, acc, pas[0], mybir.AluOpType.add
                )
                nc.vector.tensor_tensor(
                    acc, acc, pas[2], mybir.AluOpType.add
                )

                # ---- PE taps: tap-outer over this round's 4-bank psum set
                # (sets alternate per round) so LDWEIGHTS dedupes and the
                # previous round's banks are already merged. ----
                pss = pss_r[r]
                out_sb = out_pool.tile([P, T], BF16, name="out_sb")
                for n_, (ki, k) in enumerate(enumerate(PE_TAPS)):
                    for c in range(NCHK):
                        nc.tensor.matmul(
                            pss[c],
                            wt[:, g, ki, :],
                            xt[:, c * CH + OFFS[k] : c * CH + OFFS[k] + CH],
                            start=(n_ == 0),
                            stop=(n_ == NPE - 1),
                            skip_group_check=True,
                        )
                # ---- merge: out = psum + acc per chunk on DVE, store.
                # Each dma_start is pinned to one ~24GB/s queue, so stores
                # are striped: 2 per round normally, 8 (per half-chunk) for
                # the last round so the tail store finishes in ~2.7us. ----
                for c in range(NCHK):
                    sl = slice(c * CH, (c + 1) * CH)
                    nc.vector.scalar_tensor_tensor(
                        out_sb[:, sl], pss[c], 1.0, acc[:, sl],
                        mybir.AluOpType.mult, mybir.AluOpType.add,
                    )
                    if last and store_split:
                        for hlo, hhi in ((0, CH // 2), (CH // 2, CH)):
                            nc.sync.dma_start(
                                out=youts[(b, g)][:, c * CH + hlo : c * CH + hhi],
                                in_=out_sb[:, c * CH + hlo : c * CH + hhi],
                            )
                    elif (store_split and c % 2 == 1) or (
                        not store_split and c == NCHK - 1
                    ):
                        lo = 0 if not store_split else (c - 1) * CH
                        nc.sync.dma_start(
                            out=youts[(b, g)][:, lo : (c + 1) * CH],
                            in_=out_sb[:, lo : (c + 1) * CH],
                        )

    # The tile legalizer emits one LDWEIGHTS per bf16 matmul; with tap-outer
    # ordering the 4 chunk matmuls of one tap reload identical weights.
    # Drop the duplicates, migrating their waits to the next PE-queue
    # instruction.
    PE_ENG = mybir.EngineType.PE
    for fn in nc.m.functions:
        for blk in fn.blocks:
            out_insts = []
            last_key = None
            pending = []
            for inst in blk.instructions:
                tn = type(inst).__name__
                if getattr(inst, "engine", None) == PE_ENG or tn in (
                    "InstLdweights",
                    "InstMatmult",
                ):
                    if tn == "InstLdweights":
                        w = inst.ins[0]
                        key = (
                            w.memref,
                            w.offset,
                            str(w.ap),
                            str(w.dtype),
                            str(inst.perf_mode),
                            str(inst.is_transpose),
                        )
                        if key == last_key:
                            si = inst.sync_info
                            if si is not None:
                                assert not si.on_update, inst.name
                                pending.extend(si.on_wait)
                            continue  # drop duplicate
                        last_key = key
                    elif tn == "InstMatmult":
                        pass  # legalized matmuls don't clobber the array
                    else:
                        last_key = None  # unknown PE op: be conservative
                    if pending:
                        si = inst.sync_info
                        ow = list(si.on_wait) if si else []
                        ou = list(si.on_update) if si else []
                        inst.sync_info = mybir.SyncInfo(
                            on_wait=pending + ow, on_update=ou
                        )
                        pending = []
                out_insts.append(inst)
            assert not pending
            blk.instructions = out_insts

    # TRN2 ISA structs encode a single sync-wait. Split every multi-wait
    # instruction: single-wait NoOps on the same queue immediately before it
    # carry the extra waits (the sequencer blocks on each in order).
    if not waitfix:
        return nc
    nfix = [0]
    for fn in nc.m.functions:
        for blk in fn.blocks:
            out_insts = []
            for inst in blk.instructions:
                si = inst.sync_info
                if si is not None and len(si.on_wait) > 1:
                    w = list(si.on_wait)
                    for wt_ in w[:-1]:
                        nop = mybir.InstNoOp(name=f"waitfix_{nfix[0]}")
                        nfix[0] += 1
                        nop.engine = inst.engine
                        nop.sync_info = mybir.SyncInfo(
                            on_wait=[wt_], on_update=[]
                        )
                        out_insts.append(nop)
                    inst.sync_info = mybir.SyncInfo(
                        on_wait=[w[-1]], on_update=list(si.on_update)
                    )
                out_insts.append(inst)
            blk.instructions = out_insts
    return nc


def _pack_weights(filt: np.ndarray):
    fw = filt.astype(np.float32).copy()
    fw[10] += 1.0  # fold the residual into the center tap (on PE)
    dwm = np.zeros((P, NG, NPE, P), np.float32)
    for ki, k in enumerate(PE_TAPS):
        for g in range(NG):
            dwm[np.arange(P), g, ki, np.arange(P)] = fw[k, g * P : (g + 1) * P]
    fvm = np.zeros((P, NV, NG), np.float32)
    for vi, k in enumerate(DVE_TAPS + ACT_TAPS):
        for g in range(NG):
            fvm[:, vi, g] = fw[k, g * P : (g + 1) * P]
    return dwm.astype(NPBF16), fvm


def kernel(inputs: np.ndarray, filt: np.ndarray, _trace: bool = False):
    inputs = np.asarray(inputs, dtype=np.float32)
    filt = np.asarray(filt, dtype=np.float32)

    # Channel-major, zero-padded, bf16.
    xp = np.zeros((B, D, TP), NPBF16)
    xp[:, :, PADL : PADL + T] = inputs.transpose(0, 2, 1).astype(NPBF16)
    dwm, fvm = _pack_weights(filt)
    in_maps = [
        {"x": xp[c * B_LOC : (c + 1) * B_LOC], "dw": dwm, "fv": fvm}
        for c in range(NCORES)
    ]

    key = ("nc",) + tuple(sorted(_VARIANT.items()))
    if key not in _CACHE:
        _CACHE[key] = _build_bass(**_VARIANT)
    nc = _CACHE[key]
    res = run_bass_kernel_spmd(nc, in_maps, list(range(NCORES)), trace=_trace)
    ycm = np.empty((B, D, T), NPBF16)
    for core in range(NCORES):
        r = res.results[core]
        for b in range(B_LOC):
            for g in range(NG):
                ycm[core * B_LOC + b, g * P : (g + 1) * P, :] = np.asarray(
                    r[f"y_{b}_{g}"]
                )
    out = np.ascontiguousarray(ycm.transpose(0, 2, 1)).astype(np.float32)
    if _trace:
        return out, res
    return out


if __name__ == "__main__":
    rng = np.random.default_rng(0)
    xs = rng.standard_normal((B, T, D), dtype=np.float32)
    ft = rng.standard_normal((NTAPS, D), dtype=np.float32)
    out = kernel(xs, ft)
    print("ran ok", out.shape, out.dtype)
